# revision 1
# baseline (speedup 1.0000x reference)
"""ChebNet (magnetic-Laplacian ChebConv, K=2, 2 layers + linear classifier +
log_softmax) on 8 Trainium2 NeuronCores.

Strategy: 1D row-shard of the (dense) conjugated magnetic Laplacian Lc across
8 cores (512 rows each).  The Laplacian is assembled on host from the edge
list (pure input preprocessing / sharding); all matmuls, Chebyshev recursion,
biases, classifier and log_softmax run on device.

Each core keeps Lt = Lc[rows,:].T SBUF-resident as two bf16 [4096, 512]
panels (re / im) — read from HBM exactly once.  The four spmm products
(Z1 = L@X and Z2 = 2*L@Z1 - Z0 per layer) run the panels through the
TensorEngine as the moving operand (N=512) against bf16 node-major
stationary chunks.  Between products the 512-row local result is PE-
transposed and AllGather'ed in bf16 row-group rounds (256 KB payloads)
that pipeline with the consuming product's matmuls.  The Chebyshev
combination, the i*(sum Z_k W_k)+bias twist (f32r weights), classifier and
row-wise log_softmax are fused into PSUM evictions.
"""

import sys

for _p in ("/opt/trn_rl_repo",):
    if _p not in sys.path:
        sys.path.insert(0, _p)

import numpy as np
import ml_dtypes

import concourse.bass as bass
import concourse.mybir as mybir
import concourse.tile as tile
from concourse import bacc
from concourse import bass_utils
from concourse.masks import make_identity

P = 128          # partitions
F = 256          # feature width of X / hidden layers
FH = F // P      # feature halves (2)
NK = 3           # Chebyshev orders (K+1)
C = 40           # classes
N_NODES = 4096
N_CORES = 8
TWO_PI = 2.0 * np.pi

f32 = mybir.dt.float32
f32r = mybir.dt.float32r
bf16 = mybir.dt.bfloat16


# ---------------------------------------------------------------------------
# Device program
# ---------------------------------------------------------------------------

def build_nc(n_nodes=N_NODES, n_cores=N_CORES):
    KC = n_nodes // P            # contraction chunks
    SH = n_nodes // n_cores      # local rows per core
    MT = SH // P                 # local row tiles
    if MT == 4:
        ROUNDS = [(0, 2), (2, 2)]
    elif MT == 2:
        ROUNDS = [(0, 1), (1, 1)]
    else:
        ROUNDS = [(t, 1) for t in range(MT)]

    nc = bacc.Bacc("TRN2", target_bir_lowering=False, debug=False,
                   num_devices=n_cores)

    din = {}
    for nm, shp, dt in [
        ("ltr", [P, (n_nodes // P) * SH], bf16),
        ("lti", [P, (n_nodes // P) * SH], bf16),
        ("lts", [P, (n_nodes // P) * SH], bf16),
        ("xr", [P, (n_nodes // P) * F], bf16),
        ("xi", [P, (n_nodes // P) * F], bf16),
        ("xs", [P, (n_nodes // P) * F], bf16),
        ("x0tr", [P, FH * SH], f32r), ("x0ti", [P, FH * SH], f32r),
        ("w1", [P, FH * NK * FH * P], f32r), ("w2", [P, FH * NK * FH * P], f32r),
        ("wc", [P, 2 * FH * P], f32r),
        ("b1", [P, FH], f32), ("b2", [P, FH], f32), ("bc", [P, 1], f32),
    ]:
        din[nm] = nc.dram_tensor(nm, shp, dt, kind="ExternalInput").ap()
    out_d = nc.dram_tensor("out", [SH, C], f32, kind="ExternalOutput").ap()

    with tile.TileContext(nc) as tc:
        with (
            tc.tile_pool(name="const", bufs=1) as const,
            tc.tile_pool(name="lres", bufs=1) as lres,
            tc.tile_pool(name="stat", bufs=1) as stat,
            tc.tile_pool(name="ftp", bufs=1) as ftp,
            tc.tile_pool(name="stg", bufs=1) as stg,
            tc.tile_pool(name="sm", bufs=2) as sm,
            tc.tile_pool(name="ps", bufs=1, space="PSUM") as ps,
            tc.tile_pool(name="dram", bufs=1, space="DRAM") as dram,
        ):
            # ---- resident Laplacian panels (read from HBM once; the chunk
            # loads are emitted inside product 1's consumption order) --------
            ltr_sb = lres.tile([P, KC * SH], bf16, tag="ltr", bufs=1, name="ltr_sb")
            lti_sb = lres.tile([P, KC * SH], bf16, tag="lti", bufs=1, name="lti_sb")
            lts_sb = lres.tile([P, KC * SH], bf16, tag="lts", bufs=1, name="lts_sb")

            LB = 4 if KC % 4 == 0 else 1     # L chunks per load DMA

            def load_l_group(g):
                sl = slice(g * LB * SH, (g + 1) * LB * SH)
                nc.sync.dma_start(ltr_sb[:, sl], din["ltr"][:, sl])
                nc.sync.dma_start(lti_sb[:, sl], din["lti"][:, sl])
                nc.sync.dma_start(lts_sb[:, sl], din["lts"][:, sl])

            # ---- identity (no HBM traffic; needed by first boundary) -------
            ident_f = const.tile([P, P], f32)
            make_identity(nc, ident_f[:])
            ident = const.tile([P, P], f32r)
            nc.vector.tensor_copy(ident[:], ident_f[:])

            # ---- helpers ---------------------------------------------------
            def alloc_stationary(idx):
                sr = stat.tile([P, KC * F], bf16, tag="sr", bufs=1, name=f"sr{idx}")
                si = stat.tile([P, KC * F], bf16, tag="si", bufs=1, name=f"si{idx}")
                ssum = stat.tile([P, KC * F], bf16, tag="ssum", bufs=1,
                                 name=f"ssum{idx}")
                return sr, si, ssum

            def load_stat_chunk(stats, kc, src_r, src_i):
                sr, si, ssum = stats
                sl = slice(kc * F, (kc + 1) * F)
                nc.sync.dma_start(sr[:, sl], src_r)
                nc.sync.dma_start(si[:, sl], src_i)
                nc.vector.tensor_add(ssum[:, sl], sr[:, sl], si[:, sl])

            def product(stats, idx, evict, order, pre_mm=None):
                """Karatsuba complex spmm: P1 = Lr@Sr, P2 = Li@Si,
                P3 = (Lr+Li)@(Sr+Si); Zr = P1-P2, Zi = P3-P1-P2.
                6 PSUM banks; evict(p1, p2, p3) combines them."""
                sr, si, ssum = stats
                p1 = [ps.tile([P, SH], f32, tag="prod", bufs=6, name=f"p1_{idx}_{h}")
                      for h in range(FH)]
                p2 = [ps.tile([P, SH], f32, tag="prod", bufs=6, name=f"p2_{idx}_{h}")
                      for h in range(FH)]
                p3 = [ps.tile([P, SH], f32, tag="prod", bufs=6, name=f"p3_{idx}_{h}")
                      for h in range(FH)]
                for j, kc in enumerate(order):
                    if pre_mm is not None:
                        pre_mm(kc)
                    lr = ltr_sb[:, kc * SH:(kc + 1) * SH]
                    li = lti_sb[:, kc * SH:(kc + 1) * SH]
                    ls = lts_sb[:, kc * SH:(kc + 1) * SH]
                    first, last = j == 0, j == len(order) - 1
                    for h in range(FH):
                        o = kc * F + h * P
                        nc.tensor.matmul(p1[h][:], lhsT=sr[:, o:o + P], rhs=lr,
                                         start=first, stop=last)
                        nc.tensor.matmul(p2[h][:], lhsT=si[:, o:o + P], rhs=li,
                                         start=first, stop=last)
                        nc.tensor.matmul(p3[h][:], lhsT=ssum[:, o:o + P], rhs=ls,
                                         start=first, stop=last)
                evict(p1, p2, p3)

            # DVE may read at most ONE PSUM operand per op: bounce P2
            # through SBUF scratch, then combine against P1/P3.
            def evict_copy(dst_r, dst_i):
                def fn(p1, p2, p3):
                    for h in range(FH):
                        sl = slice(h * SH, (h + 1) * SH)
                        t2 = stg.tile([P, SH], f32, tag="scr", bufs=2,
                                      name=f"t2c{id(dst_r)}_{h}")
                        nc.vector.tensor_copy(t2[:], p2[h][:])
                        nc.vector.tensor_sub(dst_r[:, sl], p1[h][:], t2[:])
                        nc.vector.tensor_sub(dst_i[:, sl], p3[h][:], t2[:])
                        nc.vector.tensor_sub(dst_i[:, sl], dst_i[:, sl], p1[h][:])
                return fn

            def evict_cheb(dst_r, dst_i, z0_r, z0_i):
                """dst = 2*Z - z0 (Chebyshev T2 step), fused eviction."""
                def fn(p1, p2, p3):
                    for h in range(FH):
                        sl = slice(h * SH, (h + 1) * SH)
                        t2 = stg.tile([P, SH], f32, tag="scr", bufs=2,
                                      name=f"t2x{id(dst_r)}_{h}")
                        u = stg.tile([P, SH], f32, tag="scr2", bufs=2,
                                     name=f"u{id(dst_r)}_{h}")
                        nc.vector.tensor_copy(t2[:], p2[h][:])
                        nc.vector.tensor_sub(u[:], p1[h][:], t2[:])
                        nc.vector.scalar_tensor_tensor(
                            dst_r[:, sl], u[:], 2.0, z0_r[:, sl],
                            op0=mybir.AluOpType.mult, op1=mybir.AluOpType.subtract)
                        nc.vector.tensor_sub(u[:], p3[h][:], t2[:])
                        nc.vector.tensor_sub(u[:], u[:], p1[h][:])
                        nc.vector.scalar_tensor_tensor(
                            dst_i[:, sl], u[:], 2.0, z0_i[:, sl],
                            op0=mybir.AluOpType.mult, op1=mybir.AluOpType.subtract)
                return fn

            def gather_boundary(src_r, src_i, idx):
                """Per row-group round: PE-transpose local Z^T to node-major
                bf16, sub-AllGather it, reload the delivered global chunks.
                Sub-gathers pipeline with the next product's matmuls."""
                stats = alloc_stationary(idx)
                stage = stg.tile([P, MT * 2 * F], bf16, tag="stage", bufs=1,
                                 name=f"stage{idx}")
                order = []
                for ri, (t0, nt) in enumerate(ROUNDS):
                    for mt in range(t0, t0 + nt):
                        for ci, src in enumerate((src_r, src_i)):
                            for h in range(FH):
                                tp = ps.tile([P, P], f32r, tag="aux", bufs=2,
                                             name=f"tp{idx}_{mt}_{ci}_{h}")
                                nc.tensor.transpose(
                                    tp[:],
                                    src[:, h * SH + mt * P: h * SH + (mt + 1) * P],
                                    ident[:])
                                dst = stage[:, mt * 2 * F + ci * F + h * P:
                                            mt * 2 * F + ci * F + (h + 1) * P]
                                nc.vector.tensor_copy(dst, tp[:])
                    cc_in = dram.tile([nt * P, 2 * F], bf16, tag=f"ccin{ri}",
                                      bufs=2, name=f"ccin{idx}_{ri}")
                    cc_out = dram.tile([n_cores * nt * P, 2 * F], bf16,
                                       tag=f"ccout{ri}", bufs=2,
                                       name=f"ccout{idx}_{ri}",
                                       addr_space="Shared")
                    nc.sync.dma_start(
                        cc_in.rearrange("(t p) f -> p t f", p=P),
                        stage.rearrange("p (mt f) -> p mt f", mt=MT)
                             [:, t0:t0 + nt])
                    nc.gpsimd.collective_compute(
                        "AllGather", mybir.AluOpType.bypass,
                        replica_groups=[list(range(n_cores))],
                        ins=[cc_in.opt()], outs=[cc_out.opt()])
                    ccv = cc_out.rearrange("(c t p) f -> p c t f", p=P,
                                           c=n_cores)
                    sr, si, ssum = stats
                    for c8 in range(n_cores):
                        kc0 = c8 * MT + t0
                        sl = slice(kc0 * F, (kc0 + nt) * F)
                        nc.sync.dma_start(
                            sr[:, sl].rearrange("p (t f) -> p t f", t=nt),
                            ccv[:, c8, :, 0:F])
                        nc.sync.dma_start(
                            si[:, sl].rearrange("p (t f) -> p t f", t=nt),
                            ccv[:, c8, :, F:2 * F])
                        nc.vector.tensor_add(ssum[:, sl], sr[:, sl], si[:, sl])
                        for t in range(nt):
                            order.append(kc0 + t)
                return stats, order

            def wproduct(w_sb, b_sb, zs_r, zs_i, dst_r, dst_i, idx):
                """Y^T = (i * sum_k Z_k W_k + b)^T : Yr = -Im(S)+b, Yi = Re(S)+b."""
                for oc in range(FH):
                    s_re = ps.tile([P, SH], f32, tag="aux", bufs=2,
                                   name=f"sre{idx}_{oc}")
                    s_im = ps.tile([P, SH], f32, tag="aux", bufs=2,
                                   name=f"sim{idx}_{oc}")
                    n_mm = NK * FH
                    cnt = 0
                    for k in range(NK):
                        for fc in range(FH):
                            w_op = w_sb[:, ((fc * NK + k) * FH + oc) * P:
                                        ((fc * NK + k) * FH + oc + 1) * P]
                            zsl = slice(fc * SH, (fc + 1) * SH)
                            fl = (cnt == 0, cnt == n_mm - 1)
                            nc.tensor.matmul(s_re[:], lhsT=w_op,
                                             rhs=zs_r[k][:, zsl],
                                             start=fl[0], stop=fl[1])
                            nc.tensor.matmul(s_im[:], lhsT=w_op,
                                             rhs=zs_i[k][:, zsl],
                                             start=fl[0], stop=fl[1])
                            cnt += 1
                    osl = slice(oc * SH, (oc + 1) * SH)
                    bia = b_sb[:, oc:oc + 1]
                    nc.scalar.activation(dst_r[:, osl], s_im[:],
                                         mybir.ActivationFunctionType.Identity,
                                         bias=bia, scale=-1.0)
                    nc.scalar.activation(dst_i[:, osl], s_re[:],
                                         mybir.ActivationFunctionType.Identity,
                                         bias=bia, scale=1.0)

            # ---- layer 1 ---------------------------------------------------
            st1 = alloc_stationary(0)

            def _load_stat_span(k0, k1):
                sr, si, ssum = st1
                sl = slice(k0 * F, k1 * F)
                nc.sync.dma_start(sr[:, sl], din["xr"][:, sl])
                nc.sync.dma_start(si[:, sl], din["xi"][:, sl])
                nc.sync.dma_start(ssum[:, sl], din["xs"][:, sl])

            def _load_l_span(k0, k1):
                sl = slice(k0 * SH, k1 * SH)
                nc.sync.dma_start(ltr_sb[:, sl], din["ltr"][:, sl])
                nc.sync.dma_start(lti_sb[:, sl], din["lti"][:, sl])
                nc.sync.dma_start(lts_sb[:, sl], din["lts"][:, sl])

            def pre1(kc):
                # chunk 0 alone (earliest possible first matmul), then the
                # rest of group 0, then LB-chunk groups
                if kc == 0:
                    _load_l_span(0, 1)
                    _load_stat_span(0, 1)
                elif kc == 1 and LB > 1:
                    _load_l_span(1, LB)
                    _load_stat_span(1, LB)
                elif kc % LB == 0:
                    _load_l_span(kc, kc + LB)
                    _load_stat_span(kc, kc + LB)

            z1t_r = ftp.tile([P, FH * SH], f32r, tag="z1tr", bufs=1, name="z1t_r")
            z1t_i = ftp.tile([P, FH * SH], f32r, tag="z1ti", bufs=1, name="z1t_i")
            product(st1, 0, evict_copy(z1t_r, z1t_i), list(range(KC)),
                    pre_mm=pre1)

            # deferred constant loads — complete during product 1
            w1_sb = const.tile([P, FH * NK * FH * P], f32r)
            nc.sync.dma_start(w1_sb[:], din["w1"])
            w2_sb = const.tile([P, FH * NK * FH * P], f32r)
            nc.sync.dma_start(w2_sb[:], din["w2"])
            wc_sb = const.tile([P, 2 * FH * P], f32r)
            nc.sync.dma_start(wc_sb[:], din["wc"])
            b1_sb = const.tile([P, FH], f32)
            nc.sync.dma_start(b1_sb[:], din["b1"])
            b2_sb = const.tile([P, FH], f32)
            nc.sync.dma_start(b2_sb[:], din["b2"])
            bc_sb = const.tile([P, 1], f32)
            nc.sync.dma_start(bc_sb[:], din["bc"])
            x0t_r = ftp.tile([P, FH * SH], f32r, tag="x0tr", bufs=1, name="x0t_r")
            nc.sync.dma_start(x0t_r[:], din["x0tr"])
            x0t_i = ftp.tile([P, FH * SH], f32r, tag="x0ti", bufs=1, name="x0t_i")
            nc.sync.dma_start(x0t_i[:], din["x0ti"])

            st2, ord2 = gather_boundary(z1t_r, z1t_i, 1)

            z2t_r = ftp.tile([P, FH * SH], f32r, tag="z2tr", bufs=1, name="z2t_r")
            z2t_i = ftp.tile([P, FH * SH], f32r, tag="z2ti", bufs=1, name="z2t_i")
            product(st2, 1, evict_cheb(z2t_r, z2t_i, x0t_r, x0t_i), ord2)

            y1t_r = ftp.tile([P, FH * SH], f32r, tag="y1tr", bufs=1, name="y1t_r")
            y1t_i = ftp.tile([P, FH * SH], f32r, tag="y1ti", bufs=1, name="y1t_i")
            wproduct(w1_sb, b1_sb, [x0t_r, z1t_r, z2t_r],
                     [x0t_i, z1t_i, z2t_i], y1t_r, y1t_i, 0)

            # ---- layer 2 ---------------------------------------------------
            st3, ord3 = gather_boundary(y1t_r, y1t_i, 2)

            z1pt_r = ftp.tile([P, FH * SH], f32r, tag="z1tr", bufs=1, name="z1pt_r")
            z1pt_i = ftp.tile([P, FH * SH], f32r, tag="z1ti", bufs=1, name="z1pt_i")
            product(st3, 2, evict_copy(z1pt_r, z1pt_i), ord3)

            st4, ord4 = gather_boundary(z1pt_r, z1pt_i, 3)

            z2pt_r = ftp.tile([P, FH * SH], f32r, tag="z2tr", bufs=1, name="z2pt_r")
            z2pt_i = ftp.tile([P, FH * SH], f32r, tag="z2ti", bufs=1, name="z2pt_i")
            product(st4, 3, evict_cheb(z2pt_r, z2pt_i, y1t_r, y1t_i), ord4)

            y2t_r = ftp.tile([P, FH * SH], f32r, tag="x0tr", bufs=1, name="y2t_r")
            y2t_i = ftp.tile([P, FH * SH], f32r, tag="x0ti", bufs=1, name="y2t_i")
            wproduct(w2_sb, b2_sb, [y1t_r, z1pt_r, z2pt_r],
                     [y1t_i, z1pt_i, z2pt_i], y2t_r, y2t_i, 1)

            # ---- classifier + log_softmax ---------------------------------
            # Wc / bc are zero-padded to 128 output classes on host, so the
            # padded logit rows are exactly zero (never read past col C).
            lg = stg.tile([P, SH], f32r, tag="lg", bufs=1, name="lg")
            ps_lg = ps.tile([P, SH], f32, tag="aux", bufs=2, name="ps_lg")
            for fcp in range(2 * FH):
                src = y2t_r if fcp < FH else y2t_i
                h = fcp % FH
                nc.tensor.matmul(
                    ps_lg[:], lhsT=wc_sb[:, fcp * P:(fcp + 1) * P],
                    rhs=src[:, h * SH:(h + 1) * SH],
                    start=(fcp == 0), stop=(fcp == 2 * FH - 1))
            nc.scalar.activation(lg[:], ps_lg[:],
                                 mybir.ActivationFunctionType.Identity,
                                 bias=bc_sb[:, 0:1], scale=1.0)
            for mt in range(MT):
                tp = ps.tile([P, P], f32r, tag="aux", bufs=2, name=f"tplg{mt}")
                nc.tensor.transpose(tp[:], lg[:, mt * P:(mt + 1) * P], ident[:])
                lgt = tp[:, 0:C]
                mneg = sm.tile([P, 1], f32, tag="mneg", bufs=2, name=f"mneg{mt}")
                nc.vector.reduce_max(mneg[:], lgt, axis=mybir.AxisListType.X,
                                     negate=True)
                ex = sm.tile([P, C], f32, tag="ex", bufs=2, name=f"ex{mt}")
                ssum = sm.tile([P, 1], f32, tag="ssum", bufs=2, name=f"ssum{mt}")
                nc.scalar.activation(ex[:], lgt,
                                     mybir.ActivationFunctionType.Exp,
                                     bias=mneg[:], accum_out=ssum[:])
                lns = sm.tile([P, 1], f32, tag="lns", bufs=2, name=f"lns{mt}")
                nc.scalar.activation(lns[:], ssum[:],
                                     mybir.ActivationFunctionType.Ln)
                ot = sm.tile([P, C], f32, tag="ot", bufs=2, name=f"ot{mt}")
                nc.vector.tensor_scalar(ot[:], lgt, mneg[:], lns[:],
                                        op0=mybir.AluOpType.add,
                                        op1=mybir.AluOpType.subtract)
                nc.sync.dma_start(out_d[mt * P:(mt + 1) * P, :], ot[:])

    nc.compile()
    return nc


# ---------------------------------------------------------------------------
# Host side: Laplacian assembly + sharding
# ---------------------------------------------------------------------------

def build_lc(edges, q, edge_weight, n):
    """conj(L) of the normalized magnetic Laplacian (max_eigen=2 branch):
    conj(L) = -A_n * exp(-i*Theta).  Returns (Lr, Li) float32 [n, n]."""
    row = np.asarray(edges[0]).astype(np.int64)
    col = np.asarray(edges[1]).astype(np.int64)
    w = np.asarray(edge_weight).astype(np.float32)
    A = np.zeros((n, n), np.float32)
    np.add.at(A, (row, col), w)
    At = A.T.copy()
    A_sym = 0.5 * (A + At)
    d = A_sym.sum(axis=0)
    d[d == 0] = 1.0
    dinv = d ** -0.5
    A_n = (dinv[:, None] * A_sym) * dinv[None, :]
    Theta = (TWO_PI * np.float32(q)) * (A - At)
    Lr = -A_n * np.cos(Theta)
    Li = A_n * np.sin(Theta)
    return Lr.astype(np.float32), Li.astype(np.float32)


def make_in_maps(real, imag, edges, q, edge_weight, W1, b1, W2, b2, Wc, bc,
                 n_nodes=N_NODES, n_cores=N_CORES):
    SH = n_nodes // n_cores
    real = np.ascontiguousarray(np.asarray(real, dtype=np.float32))
    imag = np.ascontiguousarray(np.asarray(imag, dtype=np.float32))
    KC_ = n_nodes // P

    def pack_stat(a):
        # node-major [n, F] -> stationary SBUF layout [P, KC*F] bf16
        return np.ascontiguousarray(
            np.asarray(a).reshape(KC_, P, F).transpose(1, 0, 2).reshape(P, -1)
            .astype(ml_dtypes.bfloat16))

    real_bf = pack_stat(real)
    imag_bf = pack_stat(imag)
    xsum_bf = pack_stat(real + imag)
    Lr, Li = build_lc(np.asarray(edges), float(np.asarray(q)),
                      np.asarray(edge_weight), n_nodes)

    W1 = np.asarray(W1, dtype=np.float32)
    W2 = np.asarray(W2, dtype=np.float32)
    Wc = np.asarray(Wc, dtype=np.float32)
    w1p = np.ascontiguousarray(
        W1.reshape(NK, FH, P, FH, P).transpose(2, 1, 0, 3, 4).reshape(P, -1))
    w2p = np.ascontiguousarray(
        W2.reshape(NK, FH, P, FH, P).transpose(2, 1, 0, 3, 4).reshape(P, -1))
    Wc_pad = np.zeros((P, 2 * F), np.float32)
    Wc_pad[:C, :] = Wc
    wcp = np.ascontiguousarray(
        Wc_pad.T.reshape(2 * FH, P, P).transpose(1, 0, 2).reshape(P, -1))
    b1p = np.ascontiguousarray(
        np.asarray(b1, np.float32).reshape(FH, P).T)
    b2p = np.ascontiguousarray(
        np.asarray(b2, np.float32).reshape(FH, P).T)
    bcp = np.zeros((P, 1), np.float32)
    bcp[:C, 0] = np.asarray(bc, np.float32).reshape(-1)

    in_maps = []
    for c in range(n_cores):
        rows = slice(c * SH, (c + 1) * SH)
        def pack_l(a):
            # Lt [n, SH] -> panel SBUF layout [P, KC*SH] bf16
            return np.ascontiguousarray(
                a.reshape(KC_, P, SH).transpose(1, 0, 2).reshape(P, -1)
                .astype(ml_dtypes.bfloat16))

        ltr = pack_l(Lr[rows, :].T)
        lti = pack_l(Li[rows, :].T)
        lts = pack_l((Lr[rows, :] + Li[rows, :]).T)
        x0tr = np.ascontiguousarray(
            real[rows, :].T.reshape(FH, P, SH).transpose(1, 0, 2).reshape(P, -1))
        x0ti = np.ascontiguousarray(
            imag[rows, :].T.reshape(FH, P, SH).transpose(1, 0, 2).reshape(P, -1))
        in_maps.append({
            "ltr": ltr, "lti": lti, "lts": lts,
            "xr": real_bf, "xi": imag_bf, "xs": xsum_bf,
            "x0tr": x0tr, "x0ti": x0ti,
            "w1": w1p, "w2": w2p, "wc": wcp,
            "b1": b1p, "b2": b2p, "bc": bcp,
        })
    return in_maps


_NC_CACHE = {}


def _get_nc():
    if "nc" not in _NC_CACHE:
        _NC_CACHE["nc"] = build_nc()
    return _NC_CACHE["nc"]


def kernel(real, imag, edges, q, edge_weight, W1, b1, W2, b2, Wc, bc,
           _run_kwargs=None):
    in_maps = make_in_maps(real, imag, edges, q, edge_weight,
                           W1, b1, W2, b2, Wc, bc)
    nc = _get_nc()
    res = bass_utils.run_bass_kernel_spmd(
        nc, in_maps, core_ids=list(range(N_CORES)), **(_run_kwargs or {}))
    out = np.concatenate([res.results[c]["out"] for c in range(N_CORES)], axis=0)
    if _run_kwargs:
        _NC_CACHE["last_result"] = res
    return out

